# revision 54
# baseline (speedup 1.0000x reference)
"""Trainium2 Bass kernel for CDSQN (3-layer GCN + hypernetwork Q-head).

Contract: kernel(**inputs) takes the FULL unsharded inputs (numpy) and
returns the FULL [B] float32 output. Internally shards across 8
NeuronCores: nodes/edges by dst range (4000 nodes = 4 graphs per core),
GCN weights replicated, hypernetwork Wg1 sharded over the node axis.

Structure (v5):
  layer 1: gather raw-x messages (bf16, one dst tile per gather DMA, 8
           rotating buffers, 4 SWDGE queues round-robin) -> band-limited selection-
           matrix matmuls (each 128-edge chunk is host-packed so its dsts
           fall in a static window [b0_j, b0_j+W=32), so the DVE selection
           build is [128, 32] not [128, 125]; biases enter PSUM via a
           rank-1 init matmul) -> Wc1 post-multiply (GCN linearity) ->
           relu -> t2 = h1@Wc2 -> AllGather tbl2
  layer 2: same banded feature-major gather/aggregate -> relu -> PE
           identity-transpose to node-major -> pool partials
           y += h2^T c_own in PSUM (layer 3 + mean pool are one linear op)
  hypernet: z1 chunks = (head c, d-range g) with cols (d' outer, n inner),
           so the actions-weighted n-reduction needs no accumulator; the 3
           heads stack on partition thirds 0/32/64 and unstack with one
           contiguous copy each; Exp/Ln run in waves so the activation
           table loads once per wave instead of per chunk (softplus).
All hypernet weights prefetch to SBUF at start; host inputs are packed
into 2 tensors (x_tbl + one bf16 mega tensor holding the int16 index
table and fp32 payloads via bitcast) because per-execute dispatch
overhead scales ~40us per input tensor through this runtime.
"""
import sys

sys.path.insert(0, "/opt/trn_rl_repo")

import ml_dtypes
import numpy as np

import concourse.bacc as bacc
import concourse.mybir as mybir
import concourse.tile as tile
from concourse.ap import AP

# ---- problem constants (hardcoded per spec) ----
B = 32
N_PER = 1000
TOTAL = B * N_PER          # 32000
E = 512000
F = 128                    # node_feat_dim == hidden_dim
D = 64
NH = 3
EPS = 1e-6

NCORES = 8
NODES_PER_CORE = TOTAL // NCORES       # 4000
TILE_N = 125                           # dst nodes per output tile
TILES_PER_CORE = NODES_PER_CORE // TILE_N   # 32
N_SLICE = N_PER // NCORES              # 125 (hypernet n-shard per core)
W1_COLS = NH * N_SLICE * D             # 24000
W2_COLS = NH * D * D                   # 12288
W3_COLS = NH * D                       # 192
W1_GROUPS = 16                         # head-stacked d-range groups (500 cols)
W2_GROUPS = 8                          # head-stacked e-range groups (512 cols)
BF16 = ml_dtypes.bfloat16

FP = mybir.dt.float32
BF = mybir.dt.bfloat16
AF = mybir.ActivationFunctionType
OP = mybir.AluOpType

_cache = {}


def _bcast_free(ap, n):
    """Append a broadcast (step 0) innermost free dim of size n."""
    return AP(ap.tensor, ap.offset, list(ap.ap) + [[0, n]])


def _band_schedule(GT, W):
    """Static per-chunk dst-window starts (shared by all cores/tiles)."""
    span = TILE_N - W
    if GT == 1:
        return [0]
    return [min(int(round(j * span / (GT - 1))), span) for j in range(GT)]


def _mega_layout(G):
    """Column offsets inside the packed bf16 input (fewer input tensors =>
    lower per-execute dispatch overhead). int16/fp32 payloads are bitcast
    into the bf16 tensor; every field length is kept even so the fp32
    fields stay 4-byte aligned."""
    GCOLS = TILES_PER_CORE * (G // 128)
    off = {}
    c = 0
    for name, n in (("idxw", TILES_PER_CORE * (G // 16)),
                    ("dstl", GCOLS), ("normv", GCOLS),
                    ("cown", TILES_PER_CORE * B), ("iota", TILE_N + 1),
                    ("ident", F), ("wc2", F), ("wc3", F), ("wg3", W3_COLS),
                    ("bcr", 2 * F), ("actsf", 2 * N_SLICE), ("bc3f", 2),
                    ("w1s", W1_COLS), ("w2s", W2_COLS)):
        assert c % 2 == 0
        off[name] = c
        c += n + (n % 2)
    off["total"] = c
    return off


def build_program(G, W, bg1v, bg2v, bg3v, stages=99, variant=""):
    """Build the SPMD program (one NEFF, runs on all 8 cores)."""
    GCOLS16 = G // 16                   # idx cols per tile
    GT = G // 128                       # edge chunks (matmuls) per tile
    GCOLS = TILES_PER_CORE * GT         # dstl/norm cols per core
    B0 = _band_schedule(GT, W)
    gtiles = (4 if "g4" in variant else 2 if "g2" in variant else 1)  # dst tiles per gather DMA
    gbufs = {1: 8, 2: 4, 4: 2}[gtiles] * (2 if "bx" in variant else 1)
    NGATHER = TILES_PER_CORE // gtiles

    nc = bacc.Bacc("TRN2", target_bir_lowering=False, debug=False,
                   enable_asserts=False, num_devices=NCORES,
                   num_swdge_queues=(1 if "q1" in variant else
                                     2 if "q2" in variant else 4))

    # ---- per-core inputs (packed to minimize input-tensor count) ----
    MO = _mega_layout(G)
    x_tbl = nc.dram_tensor("x_tbl", [TOTAL, F], BF, kind="ExternalInput")
    mega = nc.dram_tensor("mega", [128, MO["total"]], BF, kind="ExternalInput")
    out = nc.dram_tensor("out", [B, 1], FP, kind="ExternalOutput")

    rg = [list(range(NCORES))]

    with tile.TileContext(nc) as tc:
        with tc.tile_pool(name="const", bufs=1) as cpool, \
             tc.tile_pool(name="meta", bufs=1) as mpool, \
             tc.tile_pool(name="msgs", bufs=gbufs) as gpool, \
             tc.tile_pool(name="work", bufs=4) as wpool, \
             tc.tile_pool(name="hyp", bufs=3) as hpool, \
             tc.tile_pool(name="ps_agg",
                          bufs=(2 if "p2" in variant else 3),
                          space="PSUM") as ps_agg, \
             tc.tile_pool(name="ps_feat", bufs=2, space="PSUM") as ps_feat, \
             tc.tile_pool(name="ps_hyp", bufs=2, space="PSUM") as ps_hyp, \
             tc.tile_pool(name="dram", bufs=1, space="DRAM") as dr:

            # ---- DRAM internal tensors (collective bounce buffers) ----
            ag2_in = dr.tile([NODES_PER_CORE, F], BF, tag="ag2_in", name="ag2_in")
            tbl2 = dr.tile([TOTAL, F], BF, addr_space="Shared", tag="tbl2",
                           name="tbl2")
            tbl2_loc = (dr.tile([TOTAL, F], BF, tag="tbl2loc", name="tbl2_loc")
                        if "tloc" in variant else None)
            y_in = dr.tile([F, B], FP, tag="y_in")
            y_out = dr.tile([F, B], FP, addr_space="Shared", tag="y_out")
            h1_in = dr.tile([B, W3_COLS], FP, tag="h1_in")
            h1_out = dr.tile([B, W3_COLS], FP, addr_space="Shared", tag="h1_out")

            # ---- load constants + prefetch all hypernet weights ----
            def mg(name, n):
                return mega[:, MO[name]:MO[name] + n]

            wc2_sb = cpool.tile([F, F], BF, tag="wc2")
            nc.sync.dma_start(out=wc2_sb[:], in_=mg("wc2", F))
            wc3_sb = cpool.tile([F, F], BF, tag="wc3")
            nc.sync.dma_start(out=wc3_sb[:], in_=mg("wc3", F))
            bc3_sb = cpool.tile([F, 1], FP, tag="bc3")
            nc.sync.dma_start(out=bc3_sb[:], in_=mg("bc3f", 2).bitcast(FP))
            bc1r_sb = cpool.tile([1, F], BF, tag="bc1r")
            nc.sync.dma_start(out=bc1r_sb[:],
                              in_=mega[0:1, MO["bcr"]:MO["bcr"] + F])
            bc2r_sb = cpool.tile([1, F], BF, tag="bc2r")
            nc.sync.dma_start(out=bc2r_sb[:],
                              in_=mega[0:1, MO["bcr"] + F:MO["bcr"] + 2 * F])
            iota_sb = cpool.tile([128, TILE_N], BF, tag="iota")
            nc.sync.dma_start(out=iota_sb[:], in_=mg("iota", TILE_N))
            bg1_sb = cpool.tile([128, 1], FP, tag="bg1c")
            nc.gpsimd.memset(bg1_sb[:], bg1v)
            bg2_sb = cpool.tile([128, 1], FP, tag="bg2c")
            nc.gpsimd.memset(bg2_sb[:], bg2v)
            bg3_sb = cpool.tile([128, 1], FP, tag="bg3c")
            nc.gpsimd.memset(bg3_sb[:], bg3v)
            eps_sb = cpool.tile([128, 1], FP, tag="epsc")
            nc.gpsimd.memset(eps_sb[:], EPS)
            ones_sb = cpool.tile([1, TILE_N], BF, tag="ones1")
            nc.gpsimd.memset(ones_sb[:], 1.0)
            idxw_sb = mpool.tile([128, TILES_PER_CORE * GCOLS16], mybir.dt.int16)
            nc.sync.dma_start(
                out=idxw_sb[:],
                in_=mg("idxw", TILES_PER_CORE * GCOLS16).bitcast(mybir.dt.int16))
            dstl_sb = mpool.tile([128, GCOLS], BF)
            nc.sync.dma_start(out=dstl_sb[:], in_=mg("dstl", GCOLS))
            normv_sb = mpool.tile([128, GCOLS], BF)
            nc.sync.dma_start(out=normv_sb[:], in_=mg("normv", GCOLS))
            cown_sb = mpool.tile([128, TILES_PER_CORE * B], BF)
            nc.sync.dma_start(out=cown_sb[:], in_=mg("cown", TILES_PER_CORE * B))
            acts_sb = cpool.tile([96, N_SLICE], FP, tag="acts")
            nc.sync.dma_start(
                out=acts_sb[:],
                in_=mega[0:96, MO["actsf"]:MO["actsf"] + 2 * N_SLICE]
                .bitcast(FP))
            ident_sb = cpool.tile([F, F], BF, tag="ident")
            nc.sync.dma_start(out=ident_sb[:], in_=mg("ident", F))
            wg3_sb = cpool.tile([F, W3_COLS], BF, tag="wg3")
            nc.sync.dma_start(out=wg3_sb[:], in_=mg("wg3", W3_COLS))
            w1s_sb = mpool.tile([F, W1_COLS], BF)
            w2s_sb = mpool.tile([F, W2_COLS], BF)
            weng = nc.scalar if "wq" in variant else nc.sync

            def load_weights():
                weng.dma_start(out=w1s_sb[:], in_=mg("w1s", W1_COLS))
                weng.dma_start(out=w2s_sb[:], in_=mg("w2s", W2_COLS))

            if "late" not in variant:
                load_weights()

            def gcn_gather(src_tbl, gg, queue_num=0):
                """One gather DMA covering gtiles dst tiles."""
                msgs = gpool.tile([128, gtiles * GT, F], BF, tag="msgs")
                nc.gpsimd.dma_gather(
                    out_ap=msgs[:], in_ap=src_tbl[:],
                    idxs_ap=idxw_sb[:, gg * gtiles * GCOLS16:
                                    (gg + 1) * gtiles * GCOLS16],
                    num_idxs=gtiles * G,
                    num_idxs_reg=gtiles * G, elem_size=F,
                    single_packet=False, queue_num=queue_num)
                st = wpool.tile([128, gtiles * GT, W], BF, tag="st",
                                bufs=gbufs)
                iota_bc = AP(iota_sb[:].tensor, iota_sb[:].offset,
                             [iota_sb[:].ap[0], [0, gtiles * GT], [1, W]])
                c0 = gg * gtiles * GT
                nc.vector.tensor_tensor(
                    out=st[:], in0=iota_bc,
                    in1=_bcast_free(dstl_sb[:, c0:c0 + gtiles * GT], W),
                    op=OP.is_equal)
                nc.vector.tensor_tensor(
                    out=st[:], in0=st[:],
                    in1=_bcast_free(normv_sb[:, c0:c0 + gtiles * GT], W),
                    op=OP.mult)
                return msgs, st

            def gcn_agg(msgs, st, t, bias_row):
                """Banded feature-major aggregate [F, TILE_N] for dst tile
                index t within a gather group; the per-feature bias is folded
                in via the rank-1 init matmul."""
                agg = ps_agg.tile([F, TILE_N], FP, space="PSUM", tag="agg")
                nc.tensor.matmul(agg[:], lhsT=bias_row[:],
                                 rhs=ones_sb[:], start=True, stop=False)
                for j in range(GT):
                    b0 = B0[j]
                    nc.tensor.matmul(
                        agg[:, b0:b0 + W], lhsT=msgs[:, t * GT + j, :],
                        rhs=st[:, t * GT + j, :],
                        start=False, stop=(j == GT - 1), skip_group_check=True)
                return agg

            # ---- layer 1: aggregate raw x, h1 = relu(agg), t2 = h1 @ Wc2 ----
            if stages >= 1:
                for gg in range(NGATHER):
                    msgs, st = gcn_gather(x_tbl, gg,
                                          queue_num=(0 if "q1" in variant else
                                                     gg % 2 if "q2" in variant
                                                     else gg % 4))
                    for t in range(gtiles):
                        g = gg * gtiles + t
                        agg = gcn_agg(msgs, st, t, bias_row=bc1r_sb)
                        hT = wpool.tile([F, TILE_N], BF, tag="hT")
                        nc.scalar.activation(out=hT[:], in_=agg[:], func=AF.Relu)
                        ps_t2 = ps_feat.tile([TILE_N, F], FP, space="PSUM",
                                             tag="feat")
                        nc.tensor.matmul(ps_t2[:], lhsT=hT[:], rhs=wc2_sb[:],
                                         start=True, stop=True)
                        t2sb = wpool.tile([TILE_N, F], BF, tag="t2sb")
                        nc.scalar.activation(out=t2sb[:], in_=ps_t2[:],
                                             func=AF.Identity)
                        nc.sync.dma_start(
                            out=ag2_in[g * TILE_N:(g + 1) * TILE_N, :],
                            in_=t2sb[:])
            if stages >= 2:
                if "cag" in variant:
                    npc = NODES_PER_CORE // 4
                    for c in range(4):
                        nc.gpsimd.collective_compute(
                            "AllGather", OP.bypass, replica_groups=rg,
                            ins=[ag2_in[c * npc:(c + 1) * npc, :]],
                            outs=[tbl2[c * npc * NCORES:
                                       (c + 1) * npc * NCORES, :]])
                else:
                    nc.gpsimd.collective_compute(
                        "AllGather", OP.bypass, replica_groups=rg,
                        ins=[ag2_in[:]], outs=[tbl2[:]])
                if "tloc" in variant:
                    nc.sync.dma_start(out=tbl2_loc[:], in_=tbl2[:])

            # ---- layer 2: node-major agg, h2 = relu(agg + b2), local pool
            #      partial y += h2^T c_own accumulated in PSUM ----
            if stages >= 3:
                y_ps = ps_agg.tile([F, B], FP, space="PSUM", tag="ypool", bufs=1)
                h2all = mpool.tile([128, TILES_PER_CORE, F], BF)
                for gg in range(NGATHER):
                    msgs, st = gcn_gather(
                        tbl2_loc if "tloc" in variant else tbl2, gg,
                        queue_num=(0 if "q1" in variant else
                                   gg % 2 if "q2" in variant else gg % 4))
                    for t in range(gtiles):
                        g = gg * gtiles + t
                        agg = gcn_agg(msgs, st, t, bias_row=bc2r_sb)
                        h2T = wpool.tile([F, TILE_N], BF, tag="h2T")
                        nc.scalar.activation(out=h2T[:], in_=agg[:],
                                             func=AF.Relu)
                        pt = ps_feat.tile([TILE_N, F], FP, space="PSUM",
                                          tag="feat")
                        nc.tensor.matmul(pt[:], lhsT=h2T[:], rhs=ident_sb[:],
                                         start=True, stop=True)
                        nc.scalar.activation(out=h2all[0:TILE_N, g, :],
                                             in_=pt[:], func=AF.Identity)
                for g in range(TILES_PER_CORE):
                    nc.tensor.matmul(y_ps[:], lhsT=h2all[0:TILE_N, g, :],
                                     rhs=cown_sb[0:TILE_N, g * B:(g + 1) * B],
                                     start=(g == 0), stop=(g == TILES_PER_CORE - 1))
            if "late" in variant:
                load_weights()
            if stages >= 4:
                y_sb = wpool.tile([F, B], FP, tag="ysb")
                nc.vector.tensor_copy(out=y_sb[:], in_=y_ps[:])
                nc.sync.dma_start(out=y_in[:], in_=y_sb[:])
                nc.gpsimd.collective_compute(
                    "AllReduce", OP.add, replica_groups=rg,
                    ins=[y_in[:]], outs=[y_out[:]])

            # ---- layer 3 + mean pool tail: hgT = Wc3^T y / N_PER + bc3 ----
            if stages >= 5:
                y_f = wpool.tile([F, B], FP, tag="yf")
                nc.sync.dma_start(out=y_f[:], in_=y_out[:])
                y_bf = wpool.tile([F, B], BF, tag="ybf")
                nc.vector.tensor_copy(out=y_bf[:], in_=y_f[:])
                hg_ps = ps_feat.tile([F, B], FP, space="PSUM", tag="feat")
                nc.tensor.matmul(hg_ps[:], lhsT=wc3_sb[:], rhs=y_bf[:],
                                 start=True, stop=True)
                hgT_bf = cpool.tile([F, B], BF, tag="hgTb")
                nc.scalar.activation(out=hgT_bf[:], in_=hg_ps[:], func=AF.Identity,
                                     bias=bc3_sb[:], scale=1.0 / N_PER)

            if stages < 8:
                out_t = wpool.tile([B, 1], FP, tag="qmin")
                nc.gpsimd.memset(out_t[:], 0.0)
                nc.sync.dma_start(out=out[:], in_=out_t[:])

            # ---- hypernet h1 partial (own n-slice of w1), all-reduce ----
            # z1 chunk = (head c, d-range g): cols (d' in 4, n in 125). The 3
            # heads of d-range g are stacked on partition thirds 32c..32c+32,
            # so unstacking is a contiguous [32, 64] copy per head. Exp/Ln
            # run in waves so the activation table loads once per wave.
            if stages >= 6:
                C1 = 4 * N_SLICE           # 500 cols per chunk
                HALF1 = W1_GROUPS // 2
                h1_stk = cpool.tile([96, 4 * W1_GROUPS], FP, tag="h1stk")
                a_b = AP(acts_sb[:].tensor, acts_sb[:].offset,
                         [acts_sb[:].ap[0], [0, 4], [1, N_SLICE]])
                for w in range(2):
                    # same shape as W2's ge2 so the "gew" pool bufs are shared
                    ge1 = hpool.tile([96, W2_GROUPS * 8 * D], FP, tag="gew",
                                     bufs=2)
                    for gh in range(HALF1):
                        g = w * HALF1 + gh
                        psg = ps_hyp.tile([96, C1], FP, space="PSUM", tag="psg")
                        for c in range(NH):
                            nc.tensor.matmul(
                                psg[32 * c:32 * (c + 1), :], lhsT=hgT_bf[:],
                                rhs=w1s_sb[:, (g * NH + c) * C1:
                                           (g * NH + c + 1) * C1],
                                start=True, stop=True)
                        nc.scalar.activation(
                            out=ge1[:, gh * C1:(gh + 1) * C1], in_=psg[:],
                            func=AF.Exp, bias=bg1_sb[0:96, :])
                    for gh in range(HALF1):
                        g = w * HALF1 + gh
                        gsp = hpool.tile([96, C1], FP, tag="gsp")
                        nc.scalar.activation(out=gsp[:],
                                             in_=ge1[:, gh * C1:(gh + 1) * C1],
                                             func=AF.Ln, bias=1.0)
                        nc.vector.tensor_tensor(
                            out=gsp[:].rearrange("p (d n) -> p d n", n=N_SLICE),
                            in0=gsp[:].rearrange("p (d n) -> p d n", n=N_SLICE),
                            in1=a_b, op=OP.mult)
                        nc.vector.tensor_reduce(
                            out=h1_stk[:, 4 * g:4 * (g + 1)],
                            in_=gsp[:].rearrange("p (d n) -> p d n", n=N_SLICE),
                            axis=mybir.AxisListType.X, op=OP.add)
                # unstack [96, 64] -> [32, 192] (head c -> cols 64c..64c+64)
                h1p = wpool.tile([B, W3_COLS], FP, tag="h1p")
                for c in range(NH):
                    nc.vector.tensor_copy(
                        out=h1p[:, D * c:D * (c + 1)],
                        in_=h1_stk[32 * c:32 * (c + 1), :])
                nc.sync.dma_start(out=h1_in[:], in_=h1p[:])
                nc.gpsimd.collective_compute(
                    "AllReduce", OP.add, replica_groups=rg,
                    ins=[h1_in[:]], outs=[h1_out[:]])

            # ---- tail: h1s, w2 chunks (head-stacked, d inner), w3, min ----
            # z2 chunk = (head c, e-range g): cols (e' in 8, d in 64); the
            # matching h1s block for partition third c is h1s[b, 64c:64c+64],
            # loaded directly from h1_out columns per third.
            if stages >= 7:
                h1rep = wpool.tile([96, D], FP, tag="h1rep")
                for c in range(NH):
                    nc.sync.dma_start(out=h1rep[32 * c:32 * (c + 1), :],
                                      in_=h1_out[:, D * c:D * (c + 1)])
                h1r = wpool.tile([96, D], FP, tag="h1r")
                nc.scalar.activation(out=h1r[:], in_=h1rep[:], func=AF.Relu)
                h1s = cpool.tile([96, D], FP, tag="h1s")
                nc.scalar.activation(out=h1s[:], in_=h1r[:], func=AF.Sqrt,
                                     bias=eps_sb[0:96, :])

                C2 = 8 * D                # 8 e' x 64 d cols per chunk
                ge2 = hpool.tile([96, W2_GROUPS * 8 * D], FP, tag="gew", bufs=2)
                h2_stk = cpool.tile([96, 8 * W2_GROUPS], FP, tag="h2stk")
                psg3 = ps_feat.tile([B, W3_COLS], FP, space="PSUM", tag="feat")
                nc.tensor.matmul(psg3[:], lhsT=hgT_bf[:], rhs=wg3_sb[:],
                                 start=True, stop=True)
                g3e = wpool.tile([B, W3_COLS], FP, tag="g3e")
                nc.scalar.activation(out=g3e[:], in_=psg3[:], func=AF.Exp,
                                     bias=bg3_sb[0:B, :])
                for g in range(W2_GROUPS):
                    psg = ps_hyp.tile([96, C2], FP, space="PSUM", tag="psg")
                    for c in range(NH):
                        nc.tensor.matmul(
                            psg[32 * c:32 * (c + 1), :], lhsT=hgT_bf[:],
                            rhs=w2s_sb[:, (g * NH + c) * C2:
                                       (g * NH + c + 1) * C2],
                            start=True, stop=True)
                    nc.scalar.activation(out=ge2[:, g * C2:(g + 1) * C2],
                                         in_=psg[:], func=AF.Exp,
                                         bias=bg2_sb[0:96, :])
                g3 = wpool.tile([B, W3_COLS], FP, tag="g3")
                nc.scalar.activation(out=g3[:], in_=g3e[:], func=AF.Ln, bias=1.0)
                h1b = AP(h1s[:].tensor, h1s[:].offset,
                         [h1s[:].ap[0], [0, 8], [1, D]])
                for g in range(W2_GROUPS):
                    gsp = hpool.tile([96, C2], FP, tag="gsp")
                    nc.scalar.activation(out=gsp[:],
                                         in_=ge2[:, g * C2:(g + 1) * C2],
                                         func=AF.Ln, bias=1.0)
                    nc.vector.tensor_tensor(
                        out=gsp[:].rearrange("p (e d) -> p e d", d=D),
                        in0=gsp[:].rearrange("p (e d) -> p e d", d=D),
                        in1=h1b, op=OP.mult)
                    nc.vector.tensor_reduce(
                        out=h2_stk[:, 8 * g:8 * (g + 1)],
                        in_=gsp[:].rearrange("p (e d) -> p e d", d=D),
                        axis=mybir.AxisListType.X, op=OP.add)
                # unstack [96, 64] -> [32, 192] (head c -> cols 64c..64c+64)
                h2f = wpool.tile([B, W3_COLS], FP, tag="h2f")
                for c in range(NH):
                    nc.vector.tensor_copy(
                        out=h2f[:, D * c:D * (c + 1)],
                        in_=h2_stk[32 * c:32 * (c + 1), :])
                h2r = wpool.tile([B, W3_COLS], FP, tag="h2r")
                nc.scalar.activation(out=h2r[:], in_=h2f[:], func=AF.Relu)
                h2l = wpool.tile([B, W3_COLS], FP, tag="h2l")
                nc.scalar.activation(out=h2l[:], in_=h2r[:], func=AF.Ln, bias=1.0)
                qm = wpool.tile([B, W3_COLS], FP, tag="qm")
                nc.vector.tensor_tensor(out=qm[:], in0=h2l[:], in1=g3[:],
                                        op=OP.mult)
                qh = wpool.tile([B, NH], FP, tag="qh")
                nc.vector.tensor_reduce(out=qh[:],
                                        in_=qm[:].rearrange("b (h e) -> b h e", e=D),
                                        axis=mybir.AxisListType.X, op=OP.add)
                qmin = wpool.tile([B, 1], FP, tag="qmin")
                nc.vector.tensor_reduce(out=qmin[:], in_=qh[:],
                                        axis=mybir.AxisListType.X, op=OP.min)
                nc.sync.dma_start(out=out[:], in_=qmin[:])

    nc.finalize()
    return nc


def _band_pack(dst_rel_sorted, src_sorted, norm_sorted, GT, W, B0):
    """Greedy: place dst-ascending edges of one tile into GT 128-slot chunks
    so chunk j holds only dsts in [B0[j], B0[j]+W). Returns (src, dstl_rel,
    norm) arrays of length GT*128, or None if infeasible."""
    G = GT * 128
    n = len(dst_rel_sorted)
    if n > G:
        return None
    if n == 0:
        z = np.zeros(G, np.float32)
        return np.zeros(G, np.int32), z, z
    fill = np.zeros(GT, np.int64)
    chunk_src = [np.zeros(128, np.int32) for _ in range(GT)]
    chunk_rel = [np.zeros(128, np.float32) for _ in range(GT)]
    chunk_nrm = [np.zeros(128, np.float32) for _ in range(GT)]
    b0 = np.asarray(B0)
    # boundaries of each dst value run
    change = np.flatnonzero(np.diff(dst_rel_sorted)) + 1
    starts = np.concatenate(([0], change, [n]))
    for s, e in zip(starts[:-1], starts[1:]):
        d = int(dst_rel_sorted[s])
        lo = np.searchsorted(b0, d - W, side="right")
        hi = np.searchsorted(b0, d, side="right") - 1
        pos = s
        for j in range(lo, hi + 1):
            if pos >= e:
                break
            take = min(128 - fill[j], e - pos)
            if take <= 0:
                continue
            sl = slice(fill[j], fill[j] + take)
            chunk_src[j][sl] = src_sorted[pos:pos + take]
            chunk_rel[j][sl] = dst_rel_sorted[pos:pos + take] - b0[j]
            chunk_nrm[j][sl] = norm_sorted[pos:pos + take]
            fill[j] += take
            pos += take
        if pos < e:
            return None
    return (np.concatenate(chunk_src), np.concatenate(chunk_rel),
            np.concatenate(chunk_nrm))


def _prepare(inputs, cag=False):
    """Host-side preprocessing: norms, edge sort/banded chunk packing,
    hypernet weight relayouts, per-core arrays. cag=True permutes the
    gather-table node order to chunk-major (for the chunked AllGather):
    v = k*4000 + l -> (l//1000)*8000 + k*1000 + l%1000."""
    x = np.asarray(inputs["x"], np.float32)
    edge_index = np.asarray(inputs["edge_index"])
    batch_index = np.asarray(inputs["batch_index"])
    actions = np.asarray(inputs["actions"], np.float32)
    assert np.array_equal(batch_index.astype(np.int64),
                          np.repeat(np.arange(B, dtype=np.int64), N_PER)), \
        "kernel assumes contiguous equal-size graphs"

    src = np.concatenate([edge_index[0], np.arange(TOTAL, dtype=edge_index.dtype)])
    dst = np.concatenate([edge_index[1], np.arange(TOTAL, dtype=edge_index.dtype)])
    deg = np.bincount(dst, minlength=TOTAL).astype(np.float32)
    dinv = np.where(deg > 0, 1.0 / np.sqrt(deg), 0.0).astype(np.float32)
    norm = dinv[src] * dinv[dst]

    order = np.argsort(dst, kind="stable")
    src_s = src[order].astype(np.int32)
    dst_s = dst[order].astype(np.int32)
    norm_s = norm[order]

    n_groups = NCORES * TILES_PER_CORE
    grp = dst_s // TILE_N
    counts = np.bincount(grp, minlength=n_groups)
    starts = np.zeros(n_groups + 1, np.int64)
    np.cumsum(counts, out=starts[1:])
    G = int(np.ceil(counts.max() / 128) * 128)

    # pick (G, W): escalate if the greedy band packing fails anywhere
    packed = None
    for extra_gt in (0, 1):
        GT = G // 128 + extra_gt
        for W in (32, 48, 64, 96, TILE_N):
            B0 = _band_schedule(GT, W)
            ok = True
            per_tile = []
            for gidx in range(n_groups):
                s, e = starts[gidx], starts[gidx + 1]
                r = _band_pack(dst_s[s:e] - gidx * TILE_N, src_s[s:e],
                               norm_s[s:e], GT, W, B0)
                if r is None:
                    ok = False
                    break
                per_tile.append(r)
            if ok:
                packed = (GT, W, per_tile)
                break
        if packed:
            break
    assert packed is not None, "band packing failed"
    GT, W, per_tile = packed
    G = GT * 128

    per_core = []
    for k in range(NCORES):
        src_p = np.stack([per_tile[k * TILES_PER_CORE + t][0]
                          for t in range(TILES_PER_CORE)])    # [32, G]
        rel_p = np.stack([per_tile[k * TILES_PER_CORE + t][1]
                          for t in range(TILES_PER_CORE)])
        norm_p = np.stack([per_tile[k * TILES_PER_CORE + t][2]
                           for t in range(TILES_PER_CORE)])
        flat_src = src_p.reshape(-1)
        # dma_gather idx layout: [128, cols], row 16c+p col s -> idx[s*16+p]
        idxw = np.tile(flat_src.reshape(-1, 16).T, (8, 1)).astype(np.int16)
        # dstl/norm layout: [128, cols], row p col c -> edge c*128+p
        dstl = rel_p.reshape(-1, 128).T.astype(BF16).copy()
        normv = norm_p.reshape(-1, 128).T.astype(BF16).copy()
        per_core.append((idxw, dstl, normv))

    iota = np.tile(np.arange(TILE_N, dtype=np.float32), (128, 1)).astype(BF16)

    # pool weights: c[v, g] = sum of norm over edges v -> graph g (incl loops)
    gof = src.astype(np.int64) * B + dst.astype(np.int64) // N_PER
    c_full = np.bincount(gof, weights=norm.astype(np.float64),
                         minlength=TOTAL * B).reshape(TOTAL, B).astype(np.float32)

    bg1 = np.asarray(inputs["bg1"], np.float32)
    bg2 = np.asarray(inputs["bg2"], np.float32)
    bg3 = np.asarray(inputs["bg3"], np.float32)
    assert np.all(bg1 == bg1.flat[0]) and np.all(bg2 == bg2.flat[0]) \
        and np.all(bg3 == bg3.flat[0]), "generator biases must be constant"

    x_tbl = np.ascontiguousarray(
        (x @ np.asarray(inputs["Wc1"], np.float32)).astype(BF16))
    w1_full = np.asarray(inputs["Wg1"], np.float32).reshape(F, NH, N_PER, D)
    w2_full = np.asarray(inputs["Wg2"], np.float32).reshape(F, NH, D, D)
    Wc2 = np.asarray(inputs["Wc2"], np.float32).astype(BF16)
    Wc3 = np.asarray(inputs["Wc3"], np.float32).astype(BF16)
    Wg3b = np.asarray(inputs["Wg3"], np.float32).astype(BF16)

    # w2 cols: chunk (e-range g, head c), cols (e' outer, d inner)
    w2v = (w2_full.transpose(0, 1, 3, 2)          # [F, h, e, d]
           .reshape(F, NH, 8, 8, D)               # [F, h, ec, e', d]
           .transpose(0, 2, 1, 3, 4))             # [F, ec, h, e', d]
    w2v = np.ascontiguousarray(w2v.reshape(F, W2_COLS)).astype(BF16)

    MO = _mega_layout(G)
    GCOLS = TILES_PER_CORE * (G // 128)
    in_maps = []
    for k in range(NCORES):
        idxw, dstl, normv = per_core[k]
        # w1 cols: chunk (d-range g, head c), cols (d' in 4, n in 125)
        w1n = w1_full[:, :, k * N_SLICE:(k + 1) * N_SLICE, :]  # [F, 3, 125, 64]
        w1v = np.ascontiguousarray(
            w1n.transpose(0, 1, 3, 2)        # [F, h, d, n]
            .reshape(F, NH, 16, 4, N_SLICE)  # [F, h, dg, d', n]
            .transpose(0, 2, 1, 3, 4)        # [F, dg, h, d', n]
            .reshape(F, W1_COLS)).astype(BF16)
        blk = c_full[k * NODES_PER_CORE:(k + 1) * NODES_PER_CORE]
        tmp = np.zeros((TILES_PER_CORE, 128, B), np.float32)
        tmp[:, :TILE_N, :] = blk.reshape(TILES_PER_CORE, TILE_N, B)
        cown = (tmp.transpose(1, 0, 2).reshape(128, TILES_PER_CORE * B)
                .astype(BF16))
        acts_stk = np.tile(actions[:, k * N_SLICE:(k + 1) * N_SLICE], (NH, 1))

        mega = np.zeros((128, MO["total"]), BF16)
        mega[:, MO["idxw"]:MO["idxw"] + idxw.shape[1]] = idxw.view(BF16)
        mega[:, MO["dstl"]:MO["dstl"] + GCOLS] = dstl
        mega[:, MO["normv"]:MO["normv"] + GCOLS] = normv
        mega[:, MO["cown"]:MO["cown"] + TILES_PER_CORE * B] = cown
        mega[:, MO["iota"]:MO["iota"] + TILE_N] = iota
        mega[:, MO["ident"]:MO["ident"] + F] = np.eye(F, dtype=np.float32)
        mega[:, MO["wc2"]:MO["wc2"] + F] = Wc2
        mega[:, MO["wc3"]:MO["wc3"] + F] = Wc3
        mega[:, MO["wg3"]:MO["wg3"] + W3_COLS] = Wg3b
        mega[0, MO["bcr"]:MO["bcr"] + F] = \
            np.asarray(inputs["bc1"], np.float32).astype(BF16)
        mega[0, MO["bcr"] + F:MO["bcr"] + 2 * F] = \
            np.asarray(inputs["bc2"], np.float32).astype(BF16)
        mega[0:96, MO["actsf"]:MO["actsf"] + 2 * N_SLICE] = \
            np.ascontiguousarray(acts_stk.astype(np.float32)).view(BF16)
        mega[:, MO["bc3f"]:MO["bc3f"] + 2] = \
            np.asarray(inputs["bc3"], np.float32).reshape(F, 1).view(BF16)
        mega[:, MO["w1s"]:MO["w1s"] + W1_COLS] = w1v
        mega[:, MO["w2s"]:MO["w2s"] + W2_COLS] = w2v

        in_maps.append(dict(x_tbl=x_tbl, mega=mega))
    return (G, W), float(bg1.flat[0]), float(bg2.flat[0]), float(bg3.flat[0]), \
        in_maps


class _Runner:
    """Compiled SPMD executable reusable across calls (jit cache keyed here)."""

    def __init__(self, nc):
        import jax
        from concourse import bass2jax
        from jax.experimental.shard_map import shard_map
        from jax.sharding import Mesh, PartitionSpec

        bass2jax.install_neuronx_cc_hook()
        self.jax = jax
        part_name = nc.partition_id_tensor.name if nc.partition_id_tensor else None
        in_names, out_names, out_avals, zero_outs = [], [], [], []
        for alloc in nc.m.functions[0].allocations:
            if not isinstance(alloc, mybir.MemoryLocationSet):
                continue
            name = alloc.memorylocations[0].name
            if alloc.kind == "ExternalInput":
                if name != part_name:
                    in_names.append(name)
            elif alloc.kind == "ExternalOutput":
                out_names.append(name)
                shape = tuple(alloc.tensor_shape)
                dtype = mybir.dt.np(alloc.dtype)
                out_avals.append(jax.core.ShapedArray(shape, dtype))
                zero_outs.append(np.zeros(shape, dtype))
        self.in_names, self.out_names = in_names, out_names
        self.zero_outs = zero_outs
        n_params, n_outs = len(in_names), len(out_names)

        bind_names = in_names + out_names + ([part_name] if part_name else [])

        def _body(*args):
            operands = list(args)
            if part_name:
                operands.append(bass2jax.partition_id_tensor())
            outs = bass2jax._bass_exec_p.bind(
                *operands,
                out_avals=tuple(out_avals),
                in_names=tuple(bind_names),
                out_names=tuple(out_names),
                lowering_input_output_aliases=(),
                sim_require_finite=False,
                sim_require_nnan=False,
                nc=nc,
            )
            return tuple(outs)

        devices = jax.devices()[:NCORES]
        mesh = Mesh(np.asarray(devices), ("core",))
        self.mesh = mesh
        self.PartitionSpec = PartitionSpec
        self.fn = jax.jit(
            shard_map(_body, mesh=mesh,
                      in_specs=(PartitionSpec("core"),) * (n_params + n_outs),
                      out_specs=(PartitionSpec("core"),) * n_outs,
                      check_rep=False),
            donate_argnums=tuple(range(n_params, n_params + n_outs)),
            keep_unused=True)

    def concat_inputs(self, in_maps):
        return [np.concatenate([np.asarray(m[n]) for m in in_maps], axis=0)
                for n in self.in_names]

    def run(self, concat_in):
        zeros = [np.zeros((NCORES * z.shape[0], *z.shape[1:]), z.dtype)
                 for z in self.zero_outs]
        out_arrs = self.fn(*concat_in, *zeros)
        return [np.asarray(a) for a in out_arrs]


def _get_runner(GW, bg1v, bg2v, bg3v):
    key = (GW, bg1v, bg2v, bg3v)
    if key not in _cache:
        nc = build_program(GW[0], GW[1], bg1v, bg2v, bg3v)
        _cache[key] = _Runner(nc)
    return _cache[key]


def kernel(**inputs):
    GW, bg1v, bg2v, bg3v, in_maps = _prepare(inputs)
    runner = _get_runner(GW, bg1v, bg2v, bg3v)
    outs = runner.run(runner.concat_inputs(in_maps))
    # out tensor is [NCORES*B, 1]; every core computed the full [B] result
    return outs[0].reshape(NCORES, B)[0].astype(np.float32)
